# revision 5
# baseline (speedup 1.0000x reference)
"""Causal self-attention on 8 Trainium2 NeuronCores.

Sharding: 8 cores = 4 batches x 2 head-groups (8 heads each).
Each core runs an identical SPMD program:
  - QKV projections for its head group (weights pre-transposed + bf16 on host)
  - causal attention computed in transposed-score layout S^T[s, t] so the
    AV matmul consumes P^T directly (no on-chip transposes at all)
  - softmax denominators come for free from a ones-column appended to V
  - row-sharded Wo projection produces a partial output; the two cores of a
    batch are summed on the host during unsharding.

B=4, T=2048, D=1024, H=16, dh=64.
"""

import numpy as np
import ml_dtypes

B, T, D = 4, 2048, 1024
P = 128
KD = D // P  # 8 contraction tiles for the input dim
HL = 8  # heads per core
HP = HL // 2  # head pairs per core (pair shares a 128-partition tile)
DH = 64
TCH = 512  # t-chunk (psum bank width in fp32)
NC4 = T // TCH  # 4 chunks
NTT = T // P  # 16 t-tiles

_CACHE = {}


def _split_waits(nc, mybir, limit=1):
    """walrus in this container accepts at most one sem-wait per instruction;
    hoist extra waits onto preceding NoOps on the same engine."""
    cnt = 0
    for bb in nc.main_func.blocks:
        newlist = []
        for inst in bb.instructions:
            si = inst.sync_info
            if si is not None and len(si.on_wait) > limit:
                waits = list(si.on_wait)
                extra, keep = waits[:-limit], waits[-limit:]
                for w in extra:
                    cnt += 1
                    nop = mybir.InstNoOp(name=f"WSPLIT-{cnt}")
                    nop.engine = inst.engine
                    nop.sync_info = mybir.SyncInfo(on_wait=[w], on_update=[])
                    newlist.append(nop)
                inst.sync_info = mybir.SyncInfo(
                    on_wait=keep, on_update=list(si.on_update)
                )
            newlist.append(inst)
        bb.instructions[:] = newlist
    return cnt


def _build():
    if "nc" in _CACHE:
        return _CACHE["nc"]

    from contextlib import ExitStack

    import concourse.bass as bass
    import concourse.tile as tile
    from concourse import mybir

    f32 = mybir.dt.float32
    bf = mybir.dt.bfloat16
    Exp = mybir.ActivationFunctionType.Exp

    nc = bass.Bass()
    xT = nc.declare_dram_parameter("xT", [D, T], bf, isOutput=False)
    wq = nc.declare_dram_parameter("wq", [D, HL * DH], bf, isOutput=False)
    wk = nc.declare_dram_parameter("wk", [D, HL * DH], bf, isOutput=False)
    wv = nc.declare_dram_parameter("wv", [D, HL * DH], bf, isOutput=False)
    wo = nc.declare_dram_parameter("wo", [HL * DH, D], bf, isOutput=False)
    mk = nc.declare_dram_parameter("mask", [P, P], bf, isOutput=False)
    out = nc.declare_dram_parameter("out", [T, D], f32, isOutput=True)
    # DRAM bounce buffers for the softmax-denominator partition broadcast
    rds = [
        nc.dram_tensor(f"rd{i}", [1, TCH], f32) for i in range(HP * NC4 * 2)
    ]

    with tile.TileContext(nc) as tc, ExitStack() as ctx:
        psum = ctx.enter_context(tc.tile_pool(name="psum", bufs=8, space="PSUM"))
        per = ctx.enter_context(tc.tile_pool(name="per", bufs=1))

        wq_sb = per.tile([P, KD, HL * DH], bf)
        wk_sb = per.tile([P, KD, HL * DH], bf)
        wv_sb = per.tile([P, KD, HL * DH], bf)
        wo_sb = per.tile([P, HL * DH // P, D], bf)
        mk_sb = per.tile([P, P], bf)
        qt_sb = per.tile([P, HP, T], bf)  # Q^T: [d_out_local, t]
        kt_sb = per.tile([P, HP, T], bf)  # K^T: [d_out_local, s]
        v_sb = per.tile([P, NTT, HL, 66], bf)  # V per (t-tile, head): [s, 64+1(one)]
        yt_sb = per.tile([P, HP, T], bf)  # y^T accumulated per head pair

        xT_sb, xT_free = tc.tile([P, KD, T], bf, name="xT_sb")

        # ---- phase 0: loads ----
        nc.sync.dma_start(out=mk_sb[:], in_=mk[:, :])
        for k in range(KD):
            nc.sync.dma_start(out=xT_sb[:, k, :], in_=xT[k * P : (k + 1) * P, :])
            nc.sync.dma_start(out=wq_sb[:, k, :], in_=wq[k * P : (k + 1) * P, :])
            nc.sync.dma_start(out=wk_sb[:, k, :], in_=wk[k * P : (k + 1) * P, :])
            nc.sync.dma_start(out=wv_sb[:, k, :], in_=wv[k * P : (k + 1) * P, :])
        for k in range(HL * DH // P):
            nc.sync.dma_start(out=wo_sb[:, k, :], in_=wo[k * P : (k + 1) * P, :])
        nc.vector.memset(v_sb[:, :, :, 64:65], 1.0)

        # ---- phase 1: projections ----
        # Q^T and K^T: [d_out_local(128-tile m), t] = W^T.T @ x^T
        for m in range(HP):
            for c in range(NC4):
                tsl = slice(c * TCH, (c + 1) * TCH)
                msl = slice(m * P, (m + 1) * P)
                psq = psum.tile([P, TCH], f32, tag="ps")
                for k in range(KD):
                    nc.tensor.matmul(
                        out=psq[:],
                        lhsT=wq_sb[:, k, msl],
                        rhs=xT_sb[:, k, tsl],
                        start=(k == 0),
                        stop=(k == KD - 1),
                    )
                nc.vector.tensor_copy(out=qt_sb[:, m, tsl], in_=psq[:])
                psk = psum.tile([P, TCH], f32, tag="ps")
                for k in range(KD):
                    nc.tensor.matmul(
                        out=psk[:],
                        lhsT=wk_sb[:, k, msl],
                        rhs=xT_sb[:, k, tsl],
                        start=(k == 0),
                        stop=(k == KD - 1),
                    )
                nc.vector.tensor_copy(out=kt_sb[:, m, tsl], in_=psk[:])
        # V natural layout: [t, d_out_local], then scattered per head with a
        # stride-66 free dim so each head's 64 cols + ones col is contiguous.
        for tt in range(NTT):
            psv = psum.tile([P, TCH], f32, tag="ps")
            for k in range(KD):
                nc.tensor.matmul(
                    out=psv[:],
                    lhsT=xT_sb[:, k, tt * P : (tt + 1) * P],
                    rhs=wv_sb[:, k, :],
                    start=(k == 0),
                    stop=(k == KD - 1),
                )
            nc.vector.tensor_copy(
                out=v_sb[:, tt, :, 0:64],
                in_=psv.rearrange("p (h d) -> p h d", h=HL),
            )
        xT_free()

        pt_pool = ctx.enter_context(tc.tile_pool(name="ptp", bufs=36))
        sm_pool = ctx.enter_context(tc.tile_pool(name="smp", bufs=4))
        out_pool = ctx.enter_context(tc.tile_pool(name="outp", bufs=3))

        # ---- phase 2: attention ----
        for hp in range(HP):
            for c in range(NC4):
                n_st = 4 * c + 4
                pts = {}
                for st in range(n_st):
                    kd = st - 4 * c  # >=0 on causal-diagonal s-tiles
                    lo = max(kd, 0) * P
                    for par in (0, 1):
                        rows = slice(64 * par, 64 * par + 64)
                        pss = psum.tile([P, TCH], f32, tag="ps")
                        nc.tensor.matmul(
                            out=pss[:, lo:TCH],
                            lhsT=kt_sb[rows, hp, st * P : (st + 1) * P],
                            rhs=qt_sb[rows, hp, c * TCH + lo : (c + 1) * TCH],
                            start=True,
                            stop=True,
                        )
                        pt = pt_pool.tile([P, TCH], bf, tag="pt")
                        nc.scalar.activation(
                            out=pt[:, lo:TCH],
                            in_=pss[:, lo:TCH],
                            func=Exp,
                            scale=1.0 / np.sqrt(DH),
                        )
                        if kd >= 0:
                            nc.vector.tensor_mul(
                                pt[:, lo : lo + P], pt[:, lo : lo + P], mk_sb[:]
                            )
                        pts[(st, par)] = (pt, lo)
                for par in (0, 1):
                    hl = 2 * hp + par
                    psy = psum.tile([P, TCH], f32, tag="ps")
                    for st in range(n_st):
                        pt, lo = pts[(st, par)]
                        nc.tensor.matmul(
                            out=psy[0:65, lo:TCH],
                            lhsT=v_sb[:, st, hl, 0:65],
                            rhs=pt[:, lo:TCH],
                            start=(st == 0),
                            stop=(st == n_st - 1),
                        )
                    # normalize: y^T = psy[0:64] / psy[64] (denominator row)
                    l_sb = sm_pool.tile([1, TCH], f32, tag="l")
                    nc.vector.tensor_copy(out=l_sb[:], in_=psy[64:65, :])
                    r_sb = sm_pool.tile([1, TCH], f32, tag="r")
                    nc.vector.reciprocal(out=r_sb[:], in_=l_sb[:])
                    rd = rds[(hp * NC4 + c) * 2 + par]
                    nc.sync.dma_start(out=rd[:], in_=r_sb[:])
                    rb_sb = sm_pool.tile([64, TCH], f32, tag="rb")
                    nc.sync.dma_start(
                        out=rb_sb[:],
                        in_=bass.AP(tensor=rd, offset=0, ap=[[0, 64], [1, TCH]]),
                    )
                    rows = slice(64 * par, 64 * par + 64)
                    nc.vector.tensor_mul(
                        yt_sb[rows, hp, c * TCH : (c + 1) * TCH],
                        psy[0:64, :],
                        rb_sb[:],
                    )

        # ---- phase 3: output projection (row-sharded Wo -> partial sums) ----
        for tt in range(NTT):
            ob = out_pool.tile([P, D], f32, tag="ob")
            for n2 in range(2):
                pso = psum.tile([P, TCH], f32, tag="ps")
                for k in range(HL * DH // P):
                    nc.tensor.matmul(
                        out=pso[:],
                        lhsT=yt_sb[:, k, tt * P : (tt + 1) * P],
                        rhs=wo_sb[:, k, n2 * TCH : (n2 + 1) * TCH],
                        start=(k == 0),
                        stop=(k == HL * DH // P - 1),
                    )
                nc.vector.tensor_copy(out=ob[:, n2 * TCH : (n2 + 1) * TCH], in_=pso[:])
            nc.sync.dma_start(out=out[tt * P : (tt + 1) * P, :], in_=ob[:])

    _split_waits(nc, mybir, 1)
    _CACHE["nc"] = nc
    return nc


def kernel(x, Wq, Wk, Wv, Wo):
    from concourse.bass_utils import run_bass_kernel_spmd

    nc = _build()
    bf16 = ml_dtypes.bfloat16

    band = np.tril(np.ones((P, P), np.float32)).T.astype(bf16)  # band[s,j]=s<=j
    xTs = [np.ascontiguousarray(x[b].T).astype(bf16) for b in range(B)]
    in_maps = []
    for c in range(8):
        b, hg = divmod(c, 2)
        sl = slice(512 * hg, 512 * hg + 512)
        in_maps.append(
            {
                "xT": xTs[b],
                "wq": np.ascontiguousarray(Wq[sl, :].T).astype(bf16),
                "wk": np.ascontiguousarray(Wk[sl, :].T).astype(bf16),
                "wv": np.ascontiguousarray(Wv[sl, :].T).astype(bf16),
                "wo": np.ascontiguousarray(Wo[:, sl].T).astype(bf16),
                "mask": band,
            }
        )
    res = run_bass_kernel_spmd(nc, in_maps, list(range(8)))
    _CACHE["exec_time_ns"] = res.exec_time_ns
    outp = np.empty((B, T, D), np.float32)
    for b in range(B):
        outp[b] = res.results[2 * b]["out"] + res.results[2 * b + 1]["out"]
    return outp


# revision 6
# speedup vs baseline: 1.3790x; 1.3790x over previous
"""Causal self-attention on 8 Trainium2 NeuronCores.

Sharding: 8 cores = 4 batches x 2 head-groups (8 heads each).
Each core runs an identical SPMD program:
  - QKV projections for its head group (weights pre-transposed + bf16 on host)
  - causal attention computed in transposed-score layout S^T[s, t] so the
    AV matmul consumes P^T directly (no on-chip transposes at all)
  - softmax denominators come for free from a ones-column appended to V
  - row-sharded Wo projection produces a partial output; the two cores of a
    batch are summed on the host during unsharding.

B=4, T=2048, D=1024, H=16, dh=64.
"""

import numpy as np
import ml_dtypes

B, T, D = 4, 2048, 1024
P = 128
KD = D // P  # 8 contraction tiles for the input dim
HL = 8  # heads per core
HP = HL // 2  # head pairs per core (pair shares a 128-partition tile)
DH = 64
TCH = 512  # t-chunk (psum bank width in fp32)
NC4 = T // TCH  # 4 chunks
NTT = T // P  # 16 t-tiles

_CACHE = {}


def _split_waits(nc, mybir, limit=1):
    """walrus in this container accepts at most one sem-wait per instruction;
    hoist extra waits onto preceding NoOps on the same engine."""
    cnt = 0
    for bb in nc.main_func.blocks:
        newlist = []
        for inst in bb.instructions:
            si = inst.sync_info
            if si is not None and len(si.on_wait) > limit:
                waits = list(si.on_wait)
                extra, keep = waits[:-limit], waits[-limit:]
                for w in extra:
                    cnt += 1
                    nop = mybir.InstNoOp(name=f"WSPLIT-{cnt}")
                    nop.engine = inst.engine
                    nop.sync_info = mybir.SyncInfo(on_wait=[w], on_update=[])
                    newlist.append(nop)
                inst.sync_info = mybir.SyncInfo(
                    on_wait=keep, on_update=list(si.on_update)
                )
            newlist.append(inst)
        bb.instructions[:] = newlist
    return cnt


def _build():
    if "nc" in _CACHE:
        return _CACHE["nc"]

    from contextlib import ExitStack

    import concourse.bass as bass
    import concourse.tile as tile
    from concourse import mybir

    f32 = mybir.dt.float32
    bf = mybir.dt.bfloat16
    Exp = mybir.ActivationFunctionType.Exp

    nc = bass.Bass()
    xT = nc.declare_dram_parameter("xT", [D, T], bf, isOutput=False)
    wq = nc.declare_dram_parameter("wq", [D, HL * DH], bf, isOutput=False)
    wk = nc.declare_dram_parameter("wk", [D, HL * DH], bf, isOutput=False)
    wv = nc.declare_dram_parameter("wv", [D, HL * DH], bf, isOutput=False)
    wo = nc.declare_dram_parameter("wo", [HL * DH, D], bf, isOutput=False)
    mk = nc.declare_dram_parameter("mask", [P, P], bf, isOutput=False)
    out = nc.declare_dram_parameter("out", [T, D], f32, isOutput=True)
    # DRAM bounce buffers for the softmax-denominator partition broadcast
    lds = [nc.dram_tensor(f"ld{i}", [T // 2], f32) for i in range(HP * NC4)]
    rds = [nc.dram_tensor(f"rd{i}", [T // 2], f32) for i in range(HP * NC4)]

    with tile.TileContext(nc) as tc, ExitStack() as ctx:
        psum = ctx.enter_context(tc.tile_pool(name="psum", bufs=1, space="PSUM"))
        per = ctx.enter_context(tc.tile_pool(name="per", bufs=1))

        wq_sb = per.tile([P, KD, HL * DH], bf)
        wk_sb = per.tile([P, KD, HL * DH], bf)
        wv_sb = per.tile([P, KD, HL * DH], bf)
        wo_sb = per.tile([P, HL * DH // P, D], bf)
        mk_sb = per.tile([P, P], bf)
        qt_sb = per.tile([P, HP, T], bf)  # Q^T: [d_out_local, t]
        kt_sb = per.tile([P, HP, T], bf)  # K^T: [d_out_local, s]
        v_sb = per.tile([P, NTT, HL, 66], bf)  # V per (t-tile, head): [s, 64+1(one)]
        yt_sb = per.tile([P, HP, T], bf)  # y^T accumulated per head pair

        xT_sb, xT_free = tc.tile([P, KD, T], bf, name="xT_sb")

        # ---- phase 0: loads ----
        nc.sync.dma_start(out=mk_sb[:], in_=mk[:, :])
        for k in range(KD):
            nc.sync.dma_start(out=xT_sb[:, k, :], in_=xT[k * P : (k + 1) * P, :])
            nc.sync.dma_start(out=wq_sb[:, k, :], in_=wq[k * P : (k + 1) * P, :])
            nc.sync.dma_start(out=wk_sb[:, k, :], in_=wk[k * P : (k + 1) * P, :])
            nc.sync.dma_start(out=wv_sb[:, k, :], in_=wv[k * P : (k + 1) * P, :])
        for k in range(HL * DH // P):
            nc.sync.dma_start(out=wo_sb[:, k, :], in_=wo[k * P : (k + 1) * P, :])
        nc.vector.memset(v_sb[:, :, :, 64:65], 1.0)

        # ---- phase 1: projections ----
        # Q^T and K^T: [d_out_local(128-tile m), t] = W^T.T @ x^T
        # q and k share one 2-bank psum pair per (m, chunk).
        for m in range(HP):
            for c in range(NC4):
                tsl = slice(c * TCH, (c + 1) * TCH)
                msl = slice(m * P, (m + 1) * P)
                pqk = psum.tile([P, 2 * TCH], f32, tag="ps2", bufs=3)
                for k in range(KD):
                    nc.tensor.matmul(
                        out=pqk[:, 0:TCH],
                        lhsT=wq_sb[:, k, msl],
                        rhs=xT_sb[:, k, tsl],
                        start=(k == 0),
                        stop=(k == KD - 1),
                    )
                for k in range(KD):
                    nc.tensor.matmul(
                        out=pqk[:, TCH : 2 * TCH],
                        lhsT=wk_sb[:, k, msl],
                        rhs=xT_sb[:, k, tsl],
                        start=(k == 0),
                        stop=(k == KD - 1),
                    )
                nc.vector.tensor_copy(out=qt_sb[:, m, tsl], in_=pqk[:, 0:TCH])
                nc.vector.tensor_copy(out=kt_sb[:, m, tsl], in_=pqk[:, TCH : 2 * TCH])
        # V natural layout: [t, d_out_local], then scattered per head with a
        # stride-66 free dim so each head's 64 cols + ones col is contiguous.
        for tp in range(NTT // 2):
            pv = psum.tile([P, 2 * TCH], f32, tag="ps2", bufs=3)
            for j in range(2):
                tt = 2 * tp + j
                for k in range(KD):
                    nc.tensor.matmul(
                        out=pv[:, j * TCH : (j + 1) * TCH],
                        lhsT=xT_sb[:, k, tt * P : (tt + 1) * P],
                        rhs=wv_sb[:, k, :],
                        start=(k == 0),
                        stop=(k == KD - 1),
                    )
            for j in range(2):
                tt = 2 * tp + j
                nc.vector.tensor_copy(
                    out=v_sb[:, tt, :, 0:64],
                    in_=pv[:, j * TCH : (j + 1) * TCH].rearrange(
                        "p (h d) -> p h d", h=HL
                    ),
                )
        xT_free()

        pt_pool = ctx.enter_context(tc.tile_pool(name="ptp", bufs=20))
        ysb_pool = ctx.enter_context(tc.tile_pool(name="ysbp", bufs=3))
        sm_pool = ctx.enter_context(tc.tile_pool(name="smp", bufs=4))
        out_pool = ctx.enter_context(tc.tile_pool(name="outp", bufs=3))

        # ---- phase 2: attention ----
        for hp in range(HP):
            for c in range(NC4):
                n_st = 4 * c + 4
                pts = {}
                for st in range(n_st):
                    kd = st - 4 * c  # >=0 on causal-diagonal s-tiles
                    lo = max(kd, 0) * P
                    pss = psum.tile([P, 2 * TCH], f32, tag="ps2", bufs=3)
                    for par in (0, 1):
                        rows = slice(64 * par, 64 * par + 64)
                        nc.tensor.matmul(
                            out=pss[:, par * TCH + lo : (par + 1) * TCH],
                            lhsT=kt_sb[rows, hp, st * P : (st + 1) * P],
                            rhs=qt_sb[rows, hp, c * TCH + lo : (c + 1) * TCH],
                            start=True,
                            stop=True,
                        )
                    pt = pt_pool.tile([P, 2, TCH], bf, tag="pt")
                    nc.scalar.activation(
                        out=pt[:, :, lo:TCH],
                        in_=pss.rearrange("p (a b) -> p a b", a=2)[:, :, lo:TCH],
                        func=Exp,
                        scale=1.0 / np.sqrt(DH),
                    )
                    if kd >= 0:
                        for par in (0, 1):
                            nc.vector.tensor_mul(
                                pt[:, par, lo : lo + P], pt[:, par, lo : lo + P], mk_sb[:]
                            )
                    pts[st] = (pt, lo)
                psy = psum.tile([P, 2 * TCH], f32, tag="py", bufs=1)
                for st in range(n_st):
                    pt, lo = pts[st]
                    for par in (0, 1):
                        nc.tensor.matmul(
                            out=psy[0:65, par * TCH + lo : (par + 1) * TCH],
                            lhsT=v_sb[:, st, 2 * hp + par, 0:65],
                            rhs=pt[:, par, lo:TCH],
                            start=(st == 0),
                            stop=(st == n_st - 1),
                        )
                # evict psum early, then normalize off-SBUF:
                # y^T = psy[0:64] / psy[64] (denominator row)
                ysb = ysb_pool.tile([P, 2 * TCH], f32, tag="ysb")
                nc.vector.tensor_copy(out=ysb[0:65, :], in_=psy[0:65, :])
                it = hp * NC4 + c
                ld, rd = lds[it], rds[it]
                nc.sync.dma_start(out=ld[:], in_=ysb[64:65, :])
                l128 = sm_pool.tile([P, 8], f32, tag="l128")
                nc.sync.dma_start(
                    out=l128[:], in_=bass.AP(tensor=ld, offset=0, ap=[[8, P], [1, 8]])
                )
                r128 = sm_pool.tile([P, 8], f32, tag="r128")
                nc.vector.reciprocal(out=r128[:], in_=l128[:])
                nc.sync.dma_start(
                    out=bass.AP(tensor=rd, offset=0, ap=[[8, P], [1, 8]]), in_=r128[:]
                )
                rb = sm_pool.tile([64, 2 * TCH], f32, tag="rb")
                nc.sync.dma_start(
                    out=rb[:],
                    in_=bass.AP(tensor=rd, offset=0, ap=[[0, 64], [1, 2 * TCH]]),
                )
                for par in (0, 1):
                    rows = slice(64 * par, 64 * par + 64)
                    nc.vector.tensor_mul(
                        yt_sb[rows, hp, c * TCH : (c + 1) * TCH],
                        ysb[0:64, par * TCH : (par + 1) * TCH],
                        rb[:, par * TCH : (par + 1) * TCH],
                    )

        # ---- phase 3: output projection (row-sharded Wo -> partial sums) ----
        for tt in range(NTT):
            ob = out_pool.tile([P, D], f32, tag="ob")
            po = psum.tile([P, 2 * TCH], f32, tag="ps2", bufs=3)
            for n2 in range(2):
                for k in range(HL * DH // P):
                    nc.tensor.matmul(
                        out=po[:, n2 * TCH : (n2 + 1) * TCH],
                        lhsT=yt_sb[:, k, tt * P : (tt + 1) * P],
                        rhs=wo_sb[:, k, n2 * TCH : (n2 + 1) * TCH],
                        start=(k == 0),
                        stop=(k == HL * DH // P - 1),
                    )
            nc.vector.tensor_copy(out=ob[:], in_=po[:])
            nc.sync.dma_start(out=out[tt * P : (tt + 1) * P, :], in_=ob[:])

    _split_waits(nc, mybir, 1)
    _CACHE["nc"] = nc
    return nc


def kernel(x, Wq, Wk, Wv, Wo):
    from concourse.bass_utils import run_bass_kernel_spmd

    nc = _build()
    bf16 = ml_dtypes.bfloat16

    band = np.tril(np.ones((P, P), np.float32)).T.astype(bf16)  # band[s,j]=s<=j
    xTs = [np.ascontiguousarray(x[b].T).astype(bf16) for b in range(B)]
    in_maps = []
    for c in range(8):
        b, hg = divmod(c, 2)
        sl = slice(512 * hg, 512 * hg + 512)
        in_maps.append(
            {
                "xT": xTs[b],
                "wq": np.ascontiguousarray(Wq[sl, :].T).astype(bf16),
                "wk": np.ascontiguousarray(Wk[sl, :].T).astype(bf16),
                "wv": np.ascontiguousarray(Wv[sl, :].T).astype(bf16),
                "wo": np.ascontiguousarray(Wo[:, sl].T).astype(bf16),
                "mask": band,
            }
        )
    res = run_bass_kernel_spmd(nc, in_maps, list(range(8)))
    _CACHE["exec_time_ns"] = res.exec_time_ns
    outp = np.empty((B, T, D), np.float32)
    for b in range(B):
        outp[b] = res.results[2 * b]["out"] + res.results[2 * b + 1]["out"]
    return outp


# revision 10
# speedup vs baseline: 1.3882x; 1.0067x over previous
"""Causal self-attention on 8 Trainium2 NeuronCores.

Sharding: 8 cores = 4 batches x 2 head-groups (8 heads each).
Each core runs an identical SPMD program:
  - QKV projections for its head group (weights pre-transposed + bf16 on host)
  - causal attention computed in transposed-score layout S^T[s, t] so the
    AV matmul consumes P^T directly (no on-chip transposes at all)
  - softmax denominators come for free from a ones-column appended to V
  - row-sharded Wo projection produces a partial output; the two cores of a
    batch are summed on the host during unsharding.

B=4, T=2048, D=1024, H=16, dh=64.
"""

import numpy as np
import ml_dtypes

B, T, D = 4, 2048, 1024
P = 128
KD = D // P  # 8 contraction tiles for the input dim
HL = 8  # heads per core
HP = HL // 2  # head pairs per core (pair shares a 128-partition tile)
DH = 64
TCH = 512  # t-chunk (psum bank width in fp32)
NC4 = T // TCH  # 4 chunks
NTT = T // P  # 16 t-tiles

_CACHE = {}


def _split_waits(nc, mybir, limit=1):
    """walrus in this container accepts at most one sem-wait per instruction;
    hoist extra waits onto preceding NoOps on the same engine."""
    cnt = 0
    for bb in nc.main_func.blocks:
        newlist = []
        for inst in bb.instructions:
            si = inst.sync_info
            if si is not None and len(si.on_wait) > limit:
                waits = list(si.on_wait)
                extra, keep = waits[:-limit], waits[-limit:]
                for w in extra:
                    cnt += 1
                    nop = mybir.InstNoOp(name=f"WSPLIT-{cnt}")
                    nop.engine = inst.engine
                    nop.sync_info = mybir.SyncInfo(on_wait=[w], on_update=[])
                    newlist.append(nop)
                inst.sync_info = mybir.SyncInfo(
                    on_wait=keep, on_update=list(si.on_update)
                )
            newlist.append(inst)
        bb.instructions[:] = newlist
    return cnt


def _build():
    if "nc" in _CACHE:
        return _CACHE["nc"]

    from contextlib import ExitStack

    import concourse.bass as bass
    import concourse.tile as tile
    from concourse import mybir

    f32 = mybir.dt.float32
    bf = mybir.dt.bfloat16
    Exp = mybir.ActivationFunctionType.Exp

    nc = bass.Bass()
    xT = nc.declare_dram_parameter("xT", [D, T], bf, isOutput=False)
    wq = nc.declare_dram_parameter("wq", [D, HL * DH], bf, isOutput=False)
    wk = nc.declare_dram_parameter("wk", [D, HL * DH], bf, isOutput=False)
    wv = nc.declare_dram_parameter("wv", [D, HL * DH], bf, isOutput=False)
    wo = nc.declare_dram_parameter("wo", [HL * DH, D], bf, isOutput=False)
    mk = nc.declare_dram_parameter("mask", [P, P], bf, isOutput=False)
    out = nc.declare_dram_parameter("out", [T, D], f32, isOutput=True)
    # DRAM bounce buffers for the softmax-denominator partition broadcast
    lds = [nc.dram_tensor(f"ld{i}", [T // 2], f32) for i in range(HP * NC4)]
    rds = [nc.dram_tensor(f"rd{i}", [T // 2], f32) for i in range(HP * NC4)]

    with tile.TileContext(nc) as tc, ExitStack() as ctx:
        psum = ctx.enter_context(tc.tile_pool(name="psum", bufs=1, space="PSUM"))
        per = ctx.enter_context(tc.tile_pool(name="per", bufs=1))

        wq_sb = per.tile([P, KD, HL * DH], bf)
        wk_sb = per.tile([P, KD, HL * DH], bf)
        wv_sb = per.tile([P, KD, HL * DH], bf)
        wo_sb = per.tile([P, HL * DH // P, D], bf)
        mk_sb = per.tile([P, P], bf)
        qt_sb = per.tile([P, HP, T], bf)  # Q^T: [d_out_local, t]
        kt_sb = per.tile([P, HP, T], bf)  # K^T: [d_out_local, s]
        v_sb = per.tile([P, NTT, HL, 66], bf)  # V per (t-tile, head): [s, 64+1(one)]
        yt_sb = per.tile([P, HP, T], bf)  # y^T accumulated per head pair

        xT_sb, xT_free = tc.tile([P, KD, T], bf, name="xT_sb")

        # ---- phase 0: loads ----
        nc.sync.dma_start(out=mk_sb[:], in_=mk[:, :])
        for k in range(KD):
            nc.sync.dma_start(out=xT_sb[:, k, :], in_=xT[k * P : (k + 1) * P, :])
            nc.sync.dma_start(out=wq_sb[:, k, :], in_=wq[k * P : (k + 1) * P, :])
            nc.sync.dma_start(out=wk_sb[:, k, :], in_=wk[k * P : (k + 1) * P, :])
            nc.sync.dma_start(out=wv_sb[:, k, :], in_=wv[k * P : (k + 1) * P, :])
        for k in range(HL * DH // P):
            nc.sync.dma_start(out=wo_sb[:, k, :], in_=wo[k * P : (k + 1) * P, :])
        nc.vector.memset(v_sb[:, :, :, 64:65], 1.0)

        # ---- phase 1: projections ----
        # Q^T and K^T: [d_out_local(128-tile m), t] = W^T.T @ x^T
        # q and k share one 2-bank psum pair per (m, chunk).
        for m in range(HP):
            for c in range(NC4):
                tsl = slice(c * TCH, (c + 1) * TCH)
                msl = slice(m * P, (m + 1) * P)
                pqk = psum.tile([P, 2 * TCH], f32, tag="ps2", bufs=3)
                for k in range(KD):
                    nc.tensor.matmul(
                        out=pqk[:, 0:TCH],
                        lhsT=wq_sb[:, k, msl],
                        rhs=xT_sb[:, k, tsl],
                        start=(k == 0),
                        stop=(k == KD - 1),
                    )
                for k in range(KD):
                    nc.tensor.matmul(
                        out=pqk[:, TCH : 2 * TCH],
                        lhsT=wk_sb[:, k, msl],
                        rhs=xT_sb[:, k, tsl],
                        start=(k == 0),
                        stop=(k == KD - 1),
                    )
                nc.vector.tensor_copy(out=qt_sb[:, m, tsl], in_=pqk[:, 0:TCH])
                nc.vector.tensor_copy(out=kt_sb[:, m, tsl], in_=pqk[:, TCH : 2 * TCH])
        # V natural layout: [t, d_out_local], then scattered per head with a
        # stride-66 free dim so each head's 64 cols + ones col is contiguous.
        for tp in range(NTT // 2):
            pv = psum.tile([P, 2 * TCH], f32, tag="ps2", bufs=3)
            for j in range(2):
                tt = 2 * tp + j
                for k in range(KD):
                    nc.tensor.matmul(
                        out=pv[:, j * TCH : (j + 1) * TCH],
                        lhsT=xT_sb[:, k, tt * P : (tt + 1) * P],
                        rhs=wv_sb[:, k, :],
                        start=(k == 0),
                        stop=(k == KD - 1),
                    )
            for j in range(2):
                tt = 2 * tp + j
                nc.vector.tensor_copy(
                    out=v_sb[:, tt, :, 0:64],
                    in_=pv[:, j * TCH : (j + 1) * TCH].rearrange(
                        "p (h d) -> p h d", h=HL
                    ),
                )
        xT_free()

        pt_pool = ctx.enter_context(tc.tile_pool(name="ptp", bufs=20))
        ysb_pool = ctx.enter_context(tc.tile_pool(name="ysbp", bufs=3))
        sm_pool = ctx.enter_context(tc.tile_pool(name="smp", bufs=4))
        out_pool = ctx.enter_context(tc.tile_pool(name="outp", bufs=3))

        # ---- phase 2: attention (c outer so O-proj can follow per chunk) ----
        for c in range(NC4):
            for hp in range(HP):
                n_st = 4 * c + 4
                pts = {}
                for st in range(n_st):
                    kd = st - 4 * c  # >=0 on causal-diagonal s-tiles
                    lo = max(kd, 0) * P
                    pss = psum.tile([P, 2 * TCH], f32, tag="ps2", bufs=3)
                    for par in (0, 1):
                        rows = slice(64 * par, 64 * par + 64)
                        nc.tensor.matmul(
                            out=pss[:, par * TCH + lo : (par + 1) * TCH],
                            lhsT=kt_sb[rows, hp, st * P : (st + 1) * P],
                            rhs=qt_sb[rows, hp, c * TCH + lo : (c + 1) * TCH],
                            start=True,
                            stop=True,
                        )
                    pt = pt_pool.tile([P, 2, TCH], bf, tag="pt")
                    nc.scalar.activation(
                        out=pt[:, :, lo:TCH],
                        in_=pss.rearrange("p (a b) -> p a b", a=2)[:, :, lo:TCH],
                        func=Exp,
                        scale=1.0 / np.sqrt(DH),
                    )
                    if kd >= 0:
                        for par in (0, 1):
                            nc.vector.tensor_mul(
                                pt[:, par, lo : lo + P], pt[:, par, lo : lo + P], mk_sb[:]
                            )
                    pts[st] = (pt, lo)
                psy = psum.tile([P, 2 * TCH], f32, tag="py", bufs=1)
                for st in range(n_st):
                    pt, lo = pts[st]
                    for par in (0, 1):
                        nc.tensor.matmul(
                            out=psy[0:65, par * TCH + lo : (par + 1) * TCH],
                            lhsT=v_sb[:, st, 2 * hp + par, 0:65],
                            rhs=pt[:, par, lo:TCH],
                            start=(st == 0),
                            stop=(st == n_st - 1),
                        )
                # evict psum early, then normalize off-SBUF:
                # y^T = psy[0:64] / psy[64] (denominator row)
                ysb = ysb_pool.tile([P, 2 * TCH], f32, tag="ysb")
                nc.vector.tensor_copy(out=ysb[0:65, :], in_=psy[0:65, :])
                it = hp * NC4 + c
                ld, rd = lds[it], rds[it]
                nc.sync.dma_start(out=ld[:], in_=ysb[64:65, :])
                l128 = sm_pool.tile([P, 8], f32, tag="l128")
                nc.sync.dma_start(
                    out=l128[:], in_=bass.AP(tensor=ld, offset=0, ap=[[8, P], [1, 8]])
                )
                r128 = sm_pool.tile([P, 8], f32, tag="r128")
                nc.vector.reciprocal(out=r128[:], in_=l128[:])
                nc.sync.dma_start(
                    out=bass.AP(tensor=rd, offset=0, ap=[[8, P], [1, 8]]), in_=r128[:]
                )
                rb = sm_pool.tile([64, 2 * TCH], f32, tag="rb")
                nc.sync.dma_start(
                    out=rb[:],
                    in_=bass.AP(tensor=rd, offset=0, ap=[[0, 64], [1, 2 * TCH]]),
                )
                for par in (0, 1):
                    rows = slice(64 * par, 64 * par + 64)
                    nc.vector.tensor_mul(
                        yt_sb[rows, hp, c * TCH : (c + 1) * TCH],
                        ysb[0:64, par * TCH : (par + 1) * TCH],
                        rb[:, par * TCH : (par + 1) * TCH],
                    )

            # ---- output projection for this chunk's t-tiles ----
            # (row-sharded Wo -> partial sums; k outer so each yt stationary
            # tile is loaded once for both 512-wide output halves)
            for tt in range(4 * c, 4 * c + 4):
                ob = out_pool.tile([P, D], f32, tag="ob")
                po = psum.tile([P, 2 * TCH], f32, tag="ps2", bufs=3)
                for k in range(HL * DH // P):
                    for n2 in range(2):
                        nc.tensor.matmul(
                            out=po[:, n2 * TCH : (n2 + 1) * TCH],
                            lhsT=yt_sb[:, k, tt * P : (tt + 1) * P],
                            rhs=wo_sb[:, k, n2 * TCH : (n2 + 1) * TCH],
                            start=(k == 0),
                            stop=(k == HL * DH // P - 1),
                        )
                nc.vector.tensor_copy(out=ob[:], in_=po[:])
                nc.sync.dma_start(out=out[tt * P : (tt + 1) * P, :], in_=ob[:])

    _split_waits(nc, mybir, 1)
    _CACHE["nc"] = nc
    return nc


def kernel(x, Wq, Wk, Wv, Wo):
    from concourse.bass_utils import run_bass_kernel_spmd

    nc = _build()
    bf16 = ml_dtypes.bfloat16

    band = np.tril(np.ones((P, P), np.float32)).T.astype(bf16)  # band[s,j]=s<=j
    xTs = [np.ascontiguousarray(x[b].T).astype(bf16) for b in range(B)]
    in_maps = []
    for c in range(8):
        b, hg = divmod(c, 2)
        sl = slice(512 * hg, 512 * hg + 512)
        in_maps.append(
            {
                "xT": xTs[b],
                "wq": np.ascontiguousarray(Wq[sl, :].T).astype(bf16),
                "wk": np.ascontiguousarray(Wk[sl, :].T).astype(bf16),
                "wv": np.ascontiguousarray(Wv[sl, :].T).astype(bf16),
                "wo": np.ascontiguousarray(Wo[:, sl].T).astype(bf16),
                "mask": band,
            }
        )
    res = run_bass_kernel_spmd(nc, in_maps, list(range(8)))
    _CACHE["exec_time_ns"] = res.exec_time_ns
    outp = np.empty((B, T, D), np.float32)
    for b in range(B):
        outp[b] = res.results[2 * b]["out"] + res.results[2 * b + 1]["out"]
    return outp


# revision 13
# speedup vs baseline: 1.7476x; 1.2589x over previous
"""Causal self-attention on 8 Trainium2 NeuronCores.

Sharding: 8 cores = 4 batches x 2 head-groups (8 heads each).
Each core runs an identical SPMD program:
  - QKV projections for its head group (weights pre-transposed + bf16 on host)
  - causal attention computed in transposed-score layout S^T[s, t] so the
    AV matmul consumes P^T directly (no on-chip transposes at all)
  - softmax denominators come for free from a ones-column appended to V
  - row-sharded Wo projection produces a partial output; the two cores of a
    batch are summed on the host during unsharding.

Schedule: Q is kept in two zero-padded copies (even/odd head rows) so the
QK^T matmuls run with a full K=128 contraction - every matmul in the kernel
then uses the same PE array mode (no mode-switch drains), which lets the
builder interleave QK^T, AV (lagged 2 steps behind the exp) and
projection/output-projection "filler" matmuls into one dense PE stream that
stays busy while ScalarE computes the softmax exps.

B=4, T=2048, D=1024, H=16, dh=64.
"""

import numpy as np
import ml_dtypes

B, T, D = 4, 2048, 1024
P = 128
KD = D // P  # 8 contraction tiles for the input dim
HL = 8  # heads per core
HP = HL // 2  # head pairs per core (pair shares a 128-partition tile)
DH = 64
TCH = 512  # t-chunk (psum bank width in fp32)
NC4 = T // TCH  # 4 chunks
NTT = T // P  # 16 t-tiles
AVLAG = 2  # AV trails QK^T by this many s-tiles (hides exp latency)

_CACHE = {}


def _split_waits(nc, mybir, limit=1):
    """walrus in this container accepts at most one sem-wait per instruction;
    hoist extra waits onto preceding NoOps on the same engine."""
    cnt = 0
    for bb in nc.main_func.blocks:
        newlist = []
        for inst in bb.instructions:
            si = inst.sync_info
            if si is not None and len(si.on_wait) > limit:
                waits = list(si.on_wait)
                extra, keep = waits[:-limit], waits[-limit:]
                for w in extra:
                    cnt += 1
                    nop = mybir.InstNoOp(name=f"WSPLIT-{cnt}")
                    nop.engine = inst.engine
                    nop.sync_info = mybir.SyncInfo(on_wait=[w], on_update=[])
                    newlist.append(nop)
                inst.sync_info = mybir.SyncInfo(
                    on_wait=keep, on_update=list(si.on_update)
                )
            newlist.append(inst)
        bb.instructions[:] = newlist
    return cnt


def _build():
    if "nc" in _CACHE:
        return _CACHE["nc"]

    from contextlib import ExitStack

    import concourse.bass as bass
    import concourse.tile as tile
    from concourse import mybir

    f32 = mybir.dt.float32
    bf = mybir.dt.bfloat16
    Exp = mybir.ActivationFunctionType.Exp

    nc = bass.Bass()
    xT = nc.declare_dram_parameter("xT", [D, T], bf, isOutput=False)
    wq = nc.declare_dram_parameter("wq", [D, HL * DH], bf, isOutput=False)
    wk = nc.declare_dram_parameter("wk", [D, HL * DH], bf, isOutput=False)
    wv = nc.declare_dram_parameter("wv", [D, HL * DH], bf, isOutput=False)
    wo = nc.declare_dram_parameter("wo", [HL * DH, D], bf, isOutput=False)
    mk = nc.declare_dram_parameter("mask", [P, P], bf, isOutput=False)
    out = nc.declare_dram_parameter("out", [T, D], f32, isOutput=True)
    # DRAM bounce buffers for the softmax-denominator partition broadcast
    lds = [nc.dram_tensor(f"ld{i}", [T // 2], f32) for i in range(HP * NC4)]
    rds = [nc.dram_tensor(f"rd{i}", [T // 2], f32) for i in range(HP * NC4)]

    with tile.TileContext(nc) as tc, ExitStack() as ctx:
        psum = ctx.enter_context(tc.tile_pool(name="psum", bufs=1, space="PSUM"))
        per = ctx.enter_context(tc.tile_pool(name="per", bufs=1))

        wq_sb = per.tile([P, KD, HL * DH], bf)
        wk_sb = per.tile([P, KD, HL * DH], bf)
        wv_sb = per.tile([P, KD, HL * DH], bf)
        wo_sb = per.tile([P, HL * DH // P, D], bf)
        mk_sb = per.tile([P, P], bf)
        # Q^T in two zero-padded copies: qt0 has even-head rows (0:64) live,
        # qt1 odd-head rows (64:128); the other half stays zero so QK^T can
        # contract over all 128 partitions in the standard PE mode.
        qt0_sb = per.tile([P, HP, T], bf)
        qt1_sb = per.tile([P, HP, T], bf)
        kt_sb = per.tile([P, HP, T], bf)  # K^T: [d_out_local, s]
        v_sb = per.tile([P, NTT, HL, 66], bf)  # V per (t-tile, head): [s, 64+1]
        yt_sb = per.tile([P, HP, T], bf)  # y^T accumulated per head pair

        xT_sb = per.tile([P, KD, T], bf, name="xT_sb")

        # ---- loads + zero/one fills ----
        nc.sync.dma_start(out=mk_sb[:], in_=mk[:, :])
        for k in range(KD):
            nc.sync.dma_start(out=xT_sb[:, k, :], in_=xT[k * P : (k + 1) * P, :])
            nc.sync.dma_start(out=wq_sb[:, k, :], in_=wq[k * P : (k + 1) * P, :])
        for k in range(KD):
            nc.sync.dma_start(out=wk_sb[:, k, :], in_=wk[k * P : (k + 1) * P, :])
            nc.sync.dma_start(out=wv_sb[:, k, :], in_=wv[k * P : (k + 1) * P, :])
        for k in range(HL * DH // P):
            nc.sync.dma_start(out=wo_sb[:, k, :], in_=wo[k * P : (k + 1) * P, :])
        nc.vector.memset(v_sb[:, :, :, 64:65], 1.0)
        nc.vector.memset(qt0_sb[64:P, :, :], 0.0)
        nc.vector.memset(qt1_sb[0:64, :, :], 0.0)

        pt_pool = ctx.enter_context(tc.tile_pool(name="ptp", bufs=8))
        ysb_pool = ctx.enter_context(tc.tile_pool(name="ysbp", bufs=2))
        sm_pool = ctx.enter_context(tc.tile_pool(name="smp", bufs=4))
        out_pool = ctx.enter_context(tc.tile_pool(name="outp", bufs=2))

        def gen_proj(cc):
            """QKV projections for chunk cc; yields after each matmul."""
            tsl = slice(cc * TCH, (cc + 1) * TCH)
            for m in range(HP):
                msl = slice(m * P, (m + 1) * P)
                pq = psum.tile([P, TCH], f32, tag="pp", bufs=2, name=f"pq{cc}_{m}")
                for k in range(KD):
                    nc.tensor.matmul(
                        out=pq[:],
                        lhsT=wq_sb[:, k, msl],
                        rhs=xT_sb[:, k, tsl],
                        start=(k == 0),
                        stop=(k == KD - 1),
                    )
                    yield
                nc.vector.tensor_copy(out=qt0_sb[0:64, m, tsl], in_=pq[0:64, :])
                nc.vector.tensor_copy(out=qt1_sb[64:P, m, tsl], in_=pq[64:P, :])
                pk = psum.tile([P, TCH], f32, tag="pp", bufs=2, name=f"pk{cc}_{m}")
                for k in range(KD):
                    nc.tensor.matmul(
                        out=pk[:],
                        lhsT=wk_sb[:, k, msl],
                        rhs=xT_sb[:, k, tsl],
                        start=(k == 0),
                        stop=(k == KD - 1),
                    )
                    yield
                nc.vector.tensor_copy(out=kt_sb[:, m, tsl], in_=pk[:])
            for tt in range(4 * cc, 4 * cc + 4):
                pv = psum.tile([P, TCH], f32, tag="pp", bufs=2, name=f"pv{tt}")
                for k in range(KD):
                    nc.tensor.matmul(
                        out=pv[:],
                        lhsT=xT_sb[:, k, tt * P : (tt + 1) * P],
                        rhs=wv_sb[:, k, :],
                        start=(k == 0),
                        stop=(k == KD - 1),
                    )
                    yield
                nc.vector.tensor_copy(
                    out=v_sb[:, tt, :, 0:64],
                    in_=pv.rearrange("p (h d) -> p h d", h=HL),
                )

        def gen_oproj(chunks):
            """Output projection for the given chunks; yields per matmul."""
            for c2 in chunks:
                for tt in range(4 * c2, 4 * c2 + 4):
                    ob = out_pool.tile([P, D], f32, tag="ob", name=f"ob{tt}")
                    for n2 in range(2):
                        po = psum.tile(
                            [P, TCH], f32, tag="pp", bufs=2, name=f"po{tt}_{n2}"
                        )
                        for k in range(HL * DH // P):
                            nc.tensor.matmul(
                                out=po[:],
                                lhsT=yt_sb[:, k, tt * P : (tt + 1) * P],
                                rhs=wo_sb[:, k, n2 * TCH : (n2 + 1) * TCH],
                                start=(k == 0),
                                stop=(k == HL * DH // P - 1),
                            )
                            yield
                        nc.vector.tensor_copy(
                            out=ob[:, n2 * TCH : (n2 + 1) * TCH], in_=po[:]
                        )
                    nc.sync.dma_start(out=out[tt * P : (tt + 1) * P, :], in_=ob[:])

        # projections for chunk 0 run unzipped up front (also warms the PE)
        for _ in gen_proj(0):
            pass

        # ---- attention: per chunk, all head pairs, with filler zipped in ----
        for c in range(NC4):
            n_st = 4 * c + 4
            if c < NC4 - 1:
                filler = gen_proj(c + 1)
                n_fill = HP * 2 * KD + 4 * KD
            else:
                filler = gen_oproj([0, 1, 2])
                n_fill = 12 * 2 * (HL * DH // P)
            n_steps = HP * n_st
            acc = 0.0
            fill_done = False

            for hp in range(HP):
                pts = {}
                psys = {}

                def emit_av(st, hp=hp, pts=pts, psys=psys, n_st=n_st):
                    pt, lo = pts[st]
                    for par in (0, 1):
                        if st == 0:
                            psys[par] = psum.tile(
                                [65, TCH], f32, tag="py", bufs=2, name=f"psy{par}"
                            )
                        nc.tensor.matmul(
                            out=psys[par][:, lo:TCH],
                            lhsT=v_sb[:, st, 2 * hp + par, 0:65],
                            rhs=pt[:, par, lo:TCH],
                            start=(st == 0),
                            stop=(st == n_st - 1),
                        )

                for st in range(n_st):
                    kd = st - 4 * c  # >=0 on causal-diagonal s-tiles
                    lo = max(kd, 0) * P
                    pss = psum.tile([P, 2 * TCH], f32, tag="ps2", bufs=2, name="pss")
                    for par, qt in ((0, qt0_sb), (1, qt1_sb)):
                        nc.tensor.matmul(
                            out=pss[:, par * TCH + lo : (par + 1) * TCH],
                            lhsT=kt_sb[:, hp, st * P : (st + 1) * P],
                            rhs=qt[:, hp, c * TCH + lo : (c + 1) * TCH],
                            start=True,
                            stop=True,
                        )
                    pt = pt_pool.tile([P, 2, TCH], bf, tag="pt", name="pt")
                    nc.scalar.activation(
                        out=pt[:, :, lo:TCH],
                        in_=pss.rearrange("p (a b) -> p a b", a=2)[:, :, lo:TCH],
                        func=Exp,
                        scale=1.0 / np.sqrt(DH),
                    )
                    if kd >= 0:
                        for par in (0, 1):
                            nc.vector.tensor_mul(
                                pt[:, par, lo : lo + P],
                                pt[:, par, lo : lo + P],
                                mk_sb[:],
                            )
                    pts[st] = (pt, lo)
                    if st >= AVLAG:
                        emit_av(st - AVLAG)
                    acc += n_fill / n_steps
                    while acc >= 1.0 and not fill_done:
                        try:
                            next(filler)
                        except StopIteration:
                            fill_done = True
                        acc -= 1.0
                for st in range(n_st - AVLAG, n_st):
                    emit_av(st)

                # normalize: y^T = psy[0:64] / psy[64] (denominator row)
                ysb = ysb_pool.tile([P, 2 * TCH], f32, tag="ysb", name="ysb")
                for par in (0, 1):
                    nc.vector.tensor_copy(
                        out=ysb[0:65, par * TCH : (par + 1) * TCH], in_=psys[par][:]
                    )
                it = hp * NC4 + c
                ld, rd = lds[it], rds[it]
                nc.sync.dma_start(out=ld[:], in_=ysb[64:65, :])
                l128 = sm_pool.tile([P, 8], f32, tag="l128", name="l128")
                nc.sync.dma_start(
                    out=l128[:], in_=bass.AP(tensor=ld, offset=0, ap=[[8, P], [1, 8]])
                )
                r128 = sm_pool.tile([P, 8], f32, tag="r128", name="r128")
                nc.vector.reciprocal(out=r128[:], in_=l128[:])
                nc.sync.dma_start(
                    out=bass.AP(tensor=rd, offset=0, ap=[[8, P], [1, 8]]), in_=r128[:]
                )
                rb = sm_pool.tile([64, 2 * TCH], f32, tag="rb", name="rb")
                nc.sync.dma_start(
                    out=rb[:],
                    in_=bass.AP(tensor=rd, offset=0, ap=[[0, 64], [1, 2 * TCH]]),
                )
                for par in (0, 1):
                    rows = slice(64 * par, 64 * par + 64)
                    nc.vector.tensor_mul(
                        yt_sb[rows, hp, c * TCH : (c + 1) * TCH],
                        ysb[0:64, par * TCH : (par + 1) * TCH],
                        rb[:, par * TCH : (par + 1) * TCH],
                    )
            # drain any remaining filler for this chunk
            while not fill_done:
                try:
                    next(filler)
                except StopIteration:
                    fill_done = True

        # output projection for the last chunk
        for _ in gen_oproj([3]):
            pass

    _split_waits(nc, mybir, 1)
    _CACHE["nc"] = nc
    return nc


def kernel(x, Wq, Wk, Wv, Wo):
    from concourse.bass_utils import run_bass_kernel_spmd

    nc = _build()
    bf16 = ml_dtypes.bfloat16

    band = np.tril(np.ones((P, P), np.float32)).T.astype(bf16)  # band[s,j]=s<=j
    xTs = [np.ascontiguousarray(x[b].T).astype(bf16) for b in range(B)]
    in_maps = []
    for c in range(8):
        b, hg = divmod(c, 2)
        sl = slice(512 * hg, 512 * hg + 512)
        in_maps.append(
            {
                "xT": xTs[b],
                "wq": np.ascontiguousarray(Wq[sl, :].T).astype(bf16),
                "wk": np.ascontiguousarray(Wk[sl, :].T).astype(bf16),
                "wv": np.ascontiguousarray(Wv[sl, :].T).astype(bf16),
                "wo": np.ascontiguousarray(Wo[:, sl].T).astype(bf16),
                "mask": band,
            }
        )
    res = run_bass_kernel_spmd(nc, in_maps, list(range(8)))
    _CACHE["exec_time_ns"] = res.exec_time_ns
    outp = np.empty((B, T, D), np.float32)
    for b in range(B):
        outp[b] = res.results[2 * b]["out"] + res.results[2 * b + 1]["out"]
    return outp
